# revision 29
# baseline (speedup 1.0000x reference)
"""Causal attention (B=4, N=2048, D=1024) on 8 Trainium2 NeuronCores.

v6 design (vs v5, 145us):
  * ALL THREE projections in fp8(e4m3) DoubleRow (Q and K too, not just
    V).  numpy-sim of the exact mix: quantization noise on q,k enters
    the softmax as a ~1.4% weight perturbation which stays at 4.4e-3
    max rel err (tolerance 2e-2) PROVIDED the early-row fixup computes
    its scores from true bf16 projections.  Weights premul 32 so fp8
    values sit in the normal range (|32q| < ~100 < 240 max normal);
    exp scale = (1/32)/1024 on the main path, (1/32)/16 on the fixup.
  * Parity resharding: core 2b+s owns the EVEN/ODD key tiles of batch b
    and the SAME set as query tiles -- one fp8 x upload (1 MB, layout
    [p, dchunk, tile, token]) feeds K, Q and V projections.  Slot j
    covers q-tile 2j+s with uniform limit L=2j+2; mask data kills the
    one extra future tile on s=0 cores.
  * Intra-pair split: each core projects only its 8 key tiles of K^T
    and V, exchanged via pair AllGathers (DRAM bounce, gpsimd ring).
    Gather index h = true key tiles {h, h+2, ...}; stepped-slice
    readbacks keep the program SPMD-uniform.
  * Early-row fixup: the L=2 slot (q-tile s) runs a full bf16 path for
    true keys 0..255: bf16 projections of the core's local tile 0
    (= true tile s) for K^T/V (pair-gathered) and Q^T (local).
  * All input DMAs on ONE sync/HWDGE queue in priority order; first
    matmul needs only ~0.4 MB.  Collective order: warmup (hides the
    first-begin latency behind the auto all-8 preamble barrier), gK,
    gV, gb.
"""
import sys

sys.path.insert(0, "/opt/trn_rl_repo")

from contextlib import ExitStack

import numpy as np
import ml_dtypes

import concourse.bass as bass
import concourse.mybir as mybir
import concourse.tile as tile
from concourse import bacc
from concourse.bass_utils import run_bass_kernel_spmd

B, N, D = 4, 2048, 1024
N_CORES = 8
N_SLOTS = 8
N_KTILES = 16
SCALE = 1.0 / 32.0      # 1/sqrt(D)
PRE8 = 32.0             # premul folded into all fp8 weights
PREB = 4.0              # premul folded into the bf16 fixup wq/wk
EXP_SCALE8 = SCALE / (PRE8 * PRE8)
EXP_SCALEB = SCALE / (PREB * PREB)
NEG = -1.0e9

F32 = mybir.dt.float32
BF16 = mybir.dt.bfloat16
F8 = mybir.dt.float8e4
DR = mybir.MatmulPerfMode.DoubleRow
BF = ml_dtypes.bfloat16
F8NP = ml_dtypes.float8_e4m3

PAIRS = [[0, 1], [2, 3], [4, 5], [6, 7]]

# uniform program limits per slot (key tiles 0..L-1 computed);
# slot j on core parity s covers q-tile 2j+s
LIMITS = [2, 4, 6, 8, 10, 12, 14, 16]

_NC_CACHE = {}
TRACE = False
LAST_EXEC_NS = None


def _build_nc():
    nc = bacc.Bacc(None, target_bir_lowering=False, debug=False, num_devices=8)

    # own-parity x tiles, fp8, host-permuted to [p=d%128, dchunk, tile, token]
    x8_in = nc.declare_dram_parameter("x8_in", [128, 8, 8, 128], F8, isOutput=False)
    # bf16 x of TRUE tiles 0,1 (same on both cores) for the K/V fixup
    # projections, and of the own q-tile s for the Q fixup
    xtb_in = nc.declare_dram_parameter("xtb", [2, 128, 8, 128], BF16, isOutput=False)
    xqb_in = nc.declare_dram_parameter("xqb", [128, 8, 128], BF16, isOutput=False)
    # fp8 weights [p=d%128, dchunk, ecol] = e4m3(32*W)
    wq8 = nc.declare_dram_parameter("wq8", [128, 8, 1024], F8, isOutput=False)
    wk8 = nc.declare_dram_parameter("wk8", [128, 8, 1024], F8, isOutput=False)
    wv8 = nc.declare_dram_parameter("wv8", [128, 8, 1024], F8, isOutput=False)
    # bf16 fixup weights: wqb/wkb = 4*W, wvb = 32*W
    wqb = nc.declare_dram_parameter("wqb", [128, 8, 1024], BF16, isOutput=False)
    wkb = nc.declare_dram_parameter("wkb", [128, 8, 1024], BF16, isOutput=False)
    wvb = nc.declare_dram_parameter("wvb", [128, 8, 1024], BF16, isOutput=False)
    mask_in = nc.declare_dram_parameter("mask", [128, 512], F32, isOutput=False)
    out_q = nc.declare_dram_parameter("out_q", [N_SLOTS, 128, D], BF16, isOutput=True)

    with tile.TileContext(nc) as tc, ExitStack() as top:
        consts = top.enter_context(tc.tile_pool(name="consts", bufs=1))
        kt_pool = top.enter_context(tc.tile_pool(name="ktp", bufs=1))
        v_pool = top.enter_context(tc.tile_pool(name="vp", bufs=1))
        qt_pool = top.enter_context(tc.tile_pool(name="qtp", bufs=1))
        dram = top.enter_context(tc.tile_pool(name="dram", bufs=8, space="DRAM"))

        ones8 = consts.tile([128, 2, 16], F8)
        nc.vector.memset(ones8, PRE8)
        onesb = consts.tile([128, 8], BF16)
        nc.vector.memset(onesb, PRE8)
        mask_sb = consts.tile([128, 512], F32)

        # h-indexed so gather readbacks are CONTIGUOUS (true kt = 2t+h)
        KT8 = kt_pool.tile([128, 8, 2, 8, 128], F8)  # [p=e%128, echunk, h, t, key]
        KTb = kt_pool.tile([128, 8, 256], BF16)    # bf16 true keys 0..255
        QT8 = qt_pool.tile([128, 8, 1024], F8)     # [p=e%128, echunk, qcol]
        QTb = qt_pool.tile([128, 8, 128], BF16)    # bf16 fixup q-tile (slot 0)
        V8 = v_pool.tile([128, 2, 8, D], F8)       # [p=key%128, h, t, e]
        Vb = v_pool.tile([128, 2, D], BF16)        # bf16 V true kt0/1

        # DRAM bounce buffers for the pair exchanges
        stK = dram.tile([128, 8, 1024], F8)        # own K^T half
        gK = dram.tile([2, 128, 8, 1024], F8)
        stV = dram.tile([128, 8, 1024], F8)        # own V half
        gV = dram.tile([2, 128, 8, 1024], F8)

        with ExitStack() as ph12:
            x8_pool = ph12.enter_context(tc.tile_pool(name="x8p", bufs=1))
            xt_pool = ph12.enter_context(tc.tile_pool(name="xtp", bufs=1))
            w_pool = ph12.enter_context(tc.tile_pool(name="wp", bufs=1))
            hf_pool = ph12.enter_context(tc.tile_pool(name="hf", bufs=1))
            ps_mm = ph12.enter_context(tc.tile_pool(name="ps_mm", bufs=8, space="PSUM"))

            # ---- input DMAs: ONE queue (sync/HWDGE), strict priority ----
            wk8_sb = w_pool.tile([128, 8, 1024], F8, tag="wk8")
            x8 = x8_pool.tile([128, 8, 8, 128], F8, tag="x8")  # [p,c,t,q]
            wq8_sb = w_pool.tile([128, 8, 1024], F8, tag="wq8")
            wv8_sb = w_pool.tile([128, 8, 1024], F8, tag="wv8")
            xtb = xt_pool.tile([128, 2, 8, 128], BF16, tag="xtb")
            xqb = xt_pool.tile([128, 8, 128], BF16, tag="xqb")
            wkb_sb = w_pool.tile([128, 8, 1024], BF16, tag="wkb")
            wqb_sb = w_pool.tile([128, 8, 1024], BF16, tag="wqb")
            wvb_sb = w_pool.tile([128, 8, 1024], BF16, tag="wvb")

            nc.sync.dma_start(out=wk8_sb[:, 0:2, :], in_=wk8[:, 0:2, :])
            nc.sync.dma_start(out=x8[:, 0:2], in_=x8_in[:, 0:2])
            nc.sync.dma_start(out=wk8_sb[:, 2:8, :], in_=wk8[:, 2:8, :])
            nc.sync.dma_start(out=x8[:, 2:8], in_=x8_in[:, 2:8])
            nc.sync.dma_start(out=wq8_sb, in_=wq8[:, :, :])
            nc.sync.dma_start(out=wv8_sb, in_=wv8[:, :, :])
            nc.sync.dma_start(
                out=xtb, in_=xtb_in[:].rearrange("t p c q -> p t c q"))
            nc.sync.dma_start(out=xqb, in_=xqb_in[:, :, :])
            nc.sync.dma_start(out=wkb_sb, in_=wkb[:, :, :])
            nc.sync.dma_start(out=wqb_sb, in_=wqb[:, :, :])
            nc.sync.dma_start(out=wvb_sb, in_=wvb[:, :, :])
            nc.sync.dma_start(out=mask_sb, in_=mask_in[:, :])

            vhalf = hf_pool.tile([128, 8, 1024], F8, tag="vh")
            khalf = hf_pool.tile([128, 8, 1024], F8, tag="kh")

            def kq8_proj(w_sb, out_cb):
                # fp8 DoubleRow K^T/Q^T projection of all 8 local tiles:
                # stationary weight chunk-pair shared across both 4-tile
                # moving groups; out_cb(e, h, psum) consumes the result
                for e in range(8):
                    pps = [ps_mm.tile([128, 512], F32, tag="mm", name=f"p{e}_{h}")
                           for h in range(2)]
                    for c2 in range(4):
                        for h in range(2):
                            nc.tensor.matmul(
                                pps[h],
                                w_sb[:, 2 * c2:2 * c2 + 2, e * 128:(e + 1) * 128],
                                x8[:, 2 * c2:2 * c2 + 2, 4 * h:4 * h + 4, :],
                                start=(c2 == 0), stop=(c2 == 3),
                                perf_mode=DR,
                            )
                    for h in range(2):
                        out_cb(e, h, pps[h])

            def v_half():
                # fp8 DoubleRow; stationary x chunk-pair shared by both e-halves
                for lt in range(8):
                    vps = [ps_mm.tile([128, 512], F32, tag="mm", name=f"v{lt}_{eh}")
                           for eh in range(2)]
                    for c2 in range(4):
                        for eh in range(2):
                            nc.tensor.matmul(
                                vps[eh],
                                x8[:, 2 * c2:2 * c2 + 2, lt, :],
                                wv8_sb[:, 2 * c2:2 * c2 + 2, eh * 512:(eh + 1) * 512],
                                start=(c2 == 0), stop=(c2 == 3),
                                perf_mode=DR,
                            )
                    for eh in range(2):
                        nc.vector.tensor_copy(
                            vhalf[:, lt, eh * 512:(eh + 1) * 512], vps[eh])

            def kb_proj():
                # bf16 K^T of true tiles 0,1 (fixup), computed locally on
                # both cores -- no gather needed
                for t in range(2):
                    for e in range(8):
                        bp = ps_mm.tile([128, 512], F32, tag="mm", name=f"b{t}{e}")
                        for c in range(8):
                            nc.tensor.matmul(
                                bp[:, 0:128], wkb_sb[:, c, e * 128:(e + 1) * 128],
                                xtb[:, t, c, :],
                                start=(c == 0), stop=(c == 7),
                            )
                        nc.vector.tensor_copy(
                            KTb[:, e, t * 128:(t + 1) * 128], bp[:, 0:128])

            def qb_proj():
                # bf16 Q^T of the own q-tile s (fixup)
                for e in range(8):
                    bp = ps_mm.tile([128, 512], F32, tag="mm", name=f"qb{e}")
                    for c in range(8):
                        nc.tensor.matmul(
                            bp[:, 0:128], wqb_sb[:, c, e * 128:(e + 1) * 128],
                            xqb[:, c, :],
                            start=(c == 0), stop=(c == 7),
                        )
                    nc.vector.tensor_copy(QTb[:, e, :], bp[:, 0:128])

            def vb_fix():
                # bf16 V of true tiles 0,1 (fixup)
                for t in range(2):
                    vbp = [ps_mm.tile([128, 512], F32, tag="mm", name=f"vb{t}_{eh}")
                           for eh in range(2)]
                    for c in range(8):
                        for eh in range(2):
                            nc.tensor.matmul(
                                vbp[eh], xtb[:, t, c, :],
                                wvb_sb[:, c, eh * 512:(eh + 1) * 512],
                                start=(c == 0), stop=(c == 7),
                            )
                    for eh in range(2):
                        nc.vector.tensor_copy(
                            Vb[:, t, eh * 512:(eh + 1) * 512], vbp[eh])

            # --- projections + pair exchange (collectives on gpsimd ring) ---
            # K first: S^T needs the gathered K^T earliest and the CC core
            # processes collectives strictly in issue order.
            kq8_proj(wk8_sb, lambda e, h, ps: nc.vector.tensor_copy(
                khalf[:, e, h * 512:(h + 1) * 512], ps))
            nc.gpsimd.dma_start(out=stK[:], in_=khalf)
            nc.gpsimd.collective_compute(
                "AllGather", mybir.AluOpType.bypass, replica_groups=PAIRS,
                ins=[stK.opt()], outs=[gK.opt()])
            # readback: gather index h = true key tiles {h, h+2, ...}
            for h in range(2):
                nc.sync.dma_start(
                    out=KT8[:, :, h, :, :],
                    in_=gK[h][:, :, :].rearrange("p e (t q) -> p e t q", q=128))

            def q_out(e, h, ps):
                nc.vector.tensor_copy(QT8[:, e, h * 512:(h + 1) * 512], ps)
            kq8_proj(wq8_sb, q_out)

            v_half()
            nc.gpsimd.dma_start(out=stV[:], in_=vhalf)
            nc.gpsimd.collective_compute(
                "AllGather", mybir.AluOpType.bypass, replica_groups=PAIRS,
                ins=[stV.opt()], outs=[gV.opt()])
            for h in range(2):
                nc.sync.dma_start(out=V8[:, h, :, :], in_=gV[h][:, :, :])
            kb_proj()
            vb_fix()
            qb_proj()

        # ---- attention: S^T per key tile, then AV with P^T stationary ----
        with ExitStack() as ph3:
            pt_pool = ph3.enter_context(tc.tile_pool(name="ptp", bufs=1))
            sc_pool = ph3.enter_context(tc.tile_pool(name="scp", bufs=2))
            outp = ph3.enter_context(tc.tile_pool(name="outp", bufs=2))

            PTs = [
                pt_pool.tile([128, 8, 512], F8, tag="pt1", name="PT1"),
                pt_pool.tile([128, 16, 512], F8, tag="pt2", name="PT2"),
            ]
            Pb = pt_pool.tile([128, 2, 128], BF16, tag="pb", name="Pb")

            def st_fused(ps_st):
                # one pass over key tiles; each KT stationary chunk-pair
                # serves BOTH slot groups' S^T matmuls (kt<8)
                for kt in range(16):
                    work = []   # (group, sps, w, col0, f)
                    for g in ((1, 0) if kt < 8 else (1,)):
                        Ls = LIMITS[g * 4:(g + 1) * 4]
                        f = sum(1 for L in Ls if L <= kt)
                        w = (4 - f) * 128
                        col0 = f * 128
                        sps = ps_st.tile([128, 512], F32, tag="st",
                                         name=f"s{g}_{kt}")
                        work.append((g, sps, w, col0, f))
                    for c2 in range(4):
                        for g, sps, w, col0, f in work:
                            nc.tensor.matmul(
                                sps[:, 0:w],
                                KT8[:, 2 * c2:2 * c2 + 2, kt % 2, kt // 2, :],
                                QT8[:, 2 * c2:2 * c2 + 2,
                                    g * 512 + col0: g * 512 + col0 + w],
                                start=(c2 == 0), stop=(c2 == 3),
                                perf_mode=DR,
                            )
                    for g, sps, w, col0, f in work:
                        Ls = LIMITS[g * 4:(g + 1) * 4]
                        if kt == Ls[f] - 2:
                            nc.vector.tensor_add(
                                sps[:, 0:128], sps[:, 0:128],
                                mask_sb[:, g * 256: g * 256 + 128],
                            )
                        elif kt == Ls[f] - 1:
                            nc.vector.tensor_add(
                                sps[:, 0:128], sps[:, 0:128],
                                mask_sb[:, g * 256 + 128: g * 256 + 256],
                            )
                        nc.scalar.activation(
                            PTs[g][:, kt, col0:col0 + w], sps[:, 0:w],
                            mybir.ActivationFunctionType.Exp,
                            bias=0.0, scale=EXP_SCALE8,
                        )

            def st_fix(ps_fx):
                # bf16 S^T for the fixup slot (col0 of group 0, true kt 0,1)
                for kt in range(2):
                    spb = ps_fx.tile([128, 512], F32, tag="fx", name=f"sf{kt}")
                    for c in range(8):
                        nc.tensor.matmul(
                            spb[:, 0:128], KTb[:, c, kt * 128:(kt + 1) * 128],
                            QTb[:, c, :],
                            start=(c == 0), stop=(c == 7),
                        )
                    nc.vector.tensor_add(
                        spb[:, 0:128], spb[:, 0:128],
                        mask_sb[:, kt * 128:(kt + 1) * 128],
                    )
                    nc.scalar.activation(
                        Pb[:, kt, :], spb[:, 0:128],
                        mybir.ActivationFunctionType.Exp,
                        bias=0.0, scale=EXP_SCALEB,
                    )

            with ExitStack() as st_scope:
                ps_st = st_scope.enter_context(
                    tc.tile_pool(name="ps_st", bufs=3, space="PSUM"))
                st_fused(ps_st)

            ps_o = ph3.enter_context(tc.tile_pool(name="ps_o", bufs=3, space="PSUM"))
            ps_rs = ph3.enter_context(tc.tile_pool(name="ps_rs", bufs=1, space="PSUM"))
            ps_fx = ph3.enter_context(tc.tile_pool(name="ps_fx", bufs=1, space="PSUM"))

            def av_epilogue(slot, O_ps, rs_ps):
                stats = sc_pool.tile([128, 8], F32, tag="stats", name=f"st{slot}")
                recip = stats[:, 0:1]
                nc.vector.reciprocal(recip, rs_ps)
                out_sb = outp.tile([128, D], BF16, tag="osb", name=f"ou{slot}")
                nc.vector.tensor_scalar_mul(out_sb, O_ps, recip)
                eng = nc.scalar if slot % 2 == 0 else nc.gpsimd
                eng.dma_start(out=out_q[slot][:, :], in_=out_sb)

            def av_slot(g, j):
                # fp8 DoubleRow over key-tile pairs; rowsum reuses stationary
                PT = PTs[g]
                slot = g * 4 + j
                L = LIMITS[slot]
                col = j * 128
                O_ps = ps_o.tile([128, D], F32, tag="O", name=f"O{slot}")
                rs_ps = ps_rs.tile([128, 1], F32, tag="rs", name=f"r{slot}")
                L2 = L // 2
                for t2 in range(L2):
                    pt_blk = PT[:, 2 * t2:2 * t2 + 2, col:col + 128]
                    for h in range(2):
                        nc.tensor.matmul(
                            O_ps[:, h * 512:(h + 1) * 512], pt_blk,
                            V8[:, :, t2, h * 512:(h + 1) * 512],
                            start=(t2 == 0), stop=(t2 == L2 - 1),
                            perf_mode=DR,
                        )
                    nc.tensor.matmul(
                        rs_ps, pt_blk, ones8[:, :, 0:1],
                        start=(t2 == 0), stop=(t2 == L2 - 1),
                        perf_mode=DR,
                    )
                av_epilogue(slot, O_ps, rs_ps)

            def av_fix():
                # bf16 AV for the fixup slot (slot 0, L=2)
                O_ps = ps_o.tile([128, D], F32, tag="O", name="Ofix")
                rs_ps = ps_rs.tile([128, 1], F32, tag="rs", name="rfix")
                for kt in range(2):
                    pb_blk = Pb[:, kt, :]
                    for h in range(2):
                        nc.tensor.matmul(
                            O_ps[:, h * 512:(h + 1) * 512], pb_blk,
                            Vb[:, kt, h * 512:(h + 1) * 512],
                            start=(kt == 0), stop=(kt == 1),
                        )
                    nc.tensor.matmul(
                        rs_ps, pb_blk, onesb[:, 0:1],
                        start=(kt == 0), stop=(kt == 1),
                    )
                av_epilogue(0, O_ps, rs_ps)

            # interleave big(g1)/small(g0) slots in descending L; fixup slot
            # (L=2, bf16) last so the end-of-kernel chain is shortest.
            # st_fix sits late so the small bf16 gather (gb) has time to land.
            av_slot(1, 3)
            av_slot(0, 3)
            av_slot(1, 2)
            av_slot(0, 2)
            av_slot(1, 1)
            st_fix(ps_fx)
            av_slot(0, 1)
            av_slot(1, 0)
            av_fix()

    nc.compile()
    return nc


def _masks():
    k = np.arange(128)[:, None]
    q = np.arange(128)[None, :]
    tril_t = np.where(k <= q, 0.0, NEG).astype(np.float32)  # S^T diag block
    fullneg = np.full((128, 128), NEG, np.float32)
    zeros = np.zeros((128, 128), np.float32)
    # slot j covers q-tile 2j+s with L=2j+2 key tiles: on s=0 the diagonal
    # is at kt=L-2 (kt=L-1 fully future); on s=1 kt=L-2 is fully attended
    # and the diagonal is at kt=L-1.  Same pattern for both groups.
    m_s0 = np.concatenate([tril_t, fullneg, tril_t, fullneg], axis=1)
    m_s1 = np.concatenate([zeros, tril_t, zeros, tril_t], axis=1)
    return m_s0, m_s1


def kernel(x, Wq, Wk, Wv):
    global LAST_EXEC_NS
    x = np.asarray(x, dtype=np.float32)
    Wq = np.asarray(Wq, dtype=np.float32)
    Wk = np.asarray(Wk, dtype=np.float32)
    Wv = np.asarray(Wv, dtype=np.float32)

    if "nc" not in _NC_CACHE:
        _NC_CACHE["nc"] = _build_nc()
    nc = _NC_CACHE["nc"]

    # host pre-transpose: x[b] (N, D) -> [tile, p=d%128, c=d//128, token]
    xt_f32 = np.ascontiguousarray(
        x.reshape(B, N_KTILES, 128, 8, 128).transpose(0, 1, 4, 3, 2)
    )  # [B, tile, p, c, q] f32
    x8_all = xt_f32.astype(F8NP)
    xtb_all = xt_f32.astype(BF)

    # weights [p=d%128, dchunk, ecol]; premuls folded in
    def wprep(W, premul, dt):
        return np.ascontiguousarray(
            (premul * W).reshape(8, 128, 1024).transpose(1, 0, 2).astype(dt))
    wq8_r = wprep(Wq, PRE8, F8NP)
    wk8_r = wprep(Wk, PRE8, F8NP)
    wv8_r = wprep(Wv, PRE8, F8NP)
    wqb_r = wprep(Wq, PREB, BF)
    wkb_r = wprep(Wk, PREB, BF)
    wvb_r = wprep(Wv, PRE8, BF)

    m_s0, m_s1 = _masks()
    in_maps = []
    for c in range(N_CORES):
        b, s = divmod(c, 2)
        # own tiles {s, s+2, ...} permuted to [p, c, t, q]
        x8_core = np.ascontiguousarray(
            x8_all[b, s::2].transpose(1, 2, 0, 3))
        in_maps.append({
            "x8_in": x8_core,
            "xtb": np.ascontiguousarray(xtb_all[b, 0:2]),
            "xqb": np.ascontiguousarray(xtb_all[b, s]),
            "wq8": wq8_r, "wk8": wk8_r, "wv8": wv8_r,
            "wqb": wqb_r, "wkb": wkb_r, "wvb": wvb_r,
            "mask": m_s1 if s else m_s0,
        })

    res = run_bass_kernel_spmd(nc, in_maps, list(range(N_CORES)), trace=TRACE)
    LAST_EXEC_NS = res.exec_time_ns

    out = np.empty((B, N, D), dtype=np.float32)
    for c in range(N_CORES):
        b, s = divmod(c, 2)
        oq = np.asarray(res.results[c]["out_q"], dtype=np.float32)
        for j in range(N_SLOTS):
            g = 2 * j + s
            out[b, g * 128:(g + 1) * 128, :] = oq[j]
    return out


# revision 30
# speedup vs baseline: 1.0791x; 1.0791x over previous
"""Causal attention (B=4, N=2048, D=1024) on 8 Trainium2 NeuronCores.

v6 design (vs v5, 145us):
  * ALL THREE projections in fp8(e4m3) DoubleRow (Q and K too, not just
    V).  numpy-sim of the exact mix: quantization noise on q,k enters
    the softmax as a ~1.4% weight perturbation which stays at 4.4e-3
    max rel err (tolerance 2e-2) PROVIDED the early-row fixup computes
    its scores from true bf16 projections.  Weights premul 32 so fp8
    values sit in the normal range (|32q| < ~100 < 240 max normal);
    exp scale = (1/32)/1024 on the main path, (1/32)/16 on the fixup.
  * Parity resharding: core 2b+s owns the EVEN/ODD key tiles of batch b
    and the SAME set as query tiles -- one fp8 x upload (1 MB, layout
    [p, dchunk, tile, token]) feeds K, Q and V projections.  Slot j
    covers q-tile 2j+s with uniform limit L=2j+2; mask data kills the
    one extra future tile on s=0 cores.
  * Intra-pair split: each core projects only its 8 key tiles of K^T
    and V, exchanged via pair AllGathers (DRAM bounce, gpsimd ring).
    Gather index h = true key tiles {h, h+2, ...}; stepped-slice
    readbacks keep the program SPMD-uniform.
  * Early-row fixup: the L=2 slot (q-tile s) runs a full bf16 path for
    true keys 0..255: bf16 projections of the core's local tile 0
    (= true tile s) for K^T/V (pair-gathered) and Q^T (local).
  * All input DMAs on ONE sync/HWDGE queue in priority order; first
    matmul needs only ~0.4 MB.  Collective order: warmup (hides the
    first-begin latency behind the auto all-8 preamble barrier), gK,
    gV, gb.
"""
import sys

sys.path.insert(0, "/opt/trn_rl_repo")

from contextlib import ExitStack

import numpy as np
import ml_dtypes

import concourse.bass as bass
import concourse.mybir as mybir
import concourse.tile as tile
from concourse import bacc
from concourse.bass_utils import run_bass_kernel_spmd

B, N, D = 4, 2048, 1024
N_CORES = 8
N_SLOTS = 8
N_KTILES = 16
SCALE = 1.0 / 32.0      # 1/sqrt(D)
PRE8 = 32.0             # premul folded into all fp8 weights
PREB = 4.0              # premul folded into the bf16 fixup wq/wk
EXP_SCALE8 = SCALE / (PRE8 * PRE8)
EXP_SCALEB = SCALE / (PREB * PREB)
NEG = -1.0e9

F32 = mybir.dt.float32
BF16 = mybir.dt.bfloat16
F8 = mybir.dt.float8e4
DR = mybir.MatmulPerfMode.DoubleRow
BF = ml_dtypes.bfloat16
F8NP = ml_dtypes.float8_e4m3

PAIRS = [[0, 1], [2, 3], [4, 5], [6, 7]]

# uniform program limits per slot (key tiles 0..L-1 computed);
# slot j on core parity s covers q-tile 2j+s
LIMITS = [2, 4, 6, 8, 10, 12, 14, 16]

_NC_CACHE = {}
TRACE = False
LAST_EXEC_NS = None


def _build_nc():
    nc = bacc.Bacc(None, target_bir_lowering=False, debug=False, num_devices=8)

    # own-parity x tiles, fp8, host-permuted to [p=d%128, dchunk, tile, token]
    x8_in = nc.declare_dram_parameter("x8_in", [128, 8, 8, 128], F8, isOutput=False)
    # bf16 x of TRUE tiles 0,1 (same on both cores) for the K/V fixup
    # projections, and of the own q-tile s for the Q fixup
    xtb_in = nc.declare_dram_parameter("xtb", [2, 128, 8, 128], BF16, isOutput=False)
    # fp8 weights [p=d%128, dchunk, ecol] = e4m3(32*W)
    wq8 = nc.declare_dram_parameter("wq8", [128, 8, 1024], F8, isOutput=False)
    wk8 = nc.declare_dram_parameter("wk8", [128, 8, 1024], F8, isOutput=False)
    wv8 = nc.declare_dram_parameter("wv8", [128, 8, 1024], F8, isOutput=False)
    # bf16 fixup weight: wvb = 32*W (true-precision V for early rows)
    wvb = nc.declare_dram_parameter("wvb", [128, 8, 1024], BF16, isOutput=False)
    mask_in = nc.declare_dram_parameter("mask", [128, 512], F32, isOutput=False)
    out_q = nc.declare_dram_parameter("out_q", [N_SLOTS, 128, D], BF16, isOutput=True)

    with tile.TileContext(nc) as tc, ExitStack() as top:
        consts = top.enter_context(tc.tile_pool(name="consts", bufs=1))
        kt_pool = top.enter_context(tc.tile_pool(name="ktp", bufs=1))
        v_pool = top.enter_context(tc.tile_pool(name="vp", bufs=1))
        qt_pool = top.enter_context(tc.tile_pool(name="qtp", bufs=1))
        dram = top.enter_context(tc.tile_pool(name="dram", bufs=8, space="DRAM"))

        ones8 = consts.tile([128, 2, 16], F8)
        nc.vector.memset(ones8, PRE8)
        onesb = consts.tile([128, 8], BF16)
        nc.vector.memset(onesb, PRE8)
        mask_sb = consts.tile([128, 512], F32)

        # h-indexed so gather readbacks are CONTIGUOUS (true kt = 2t+h)
        KT8 = kt_pool.tile([128, 8, 2, 8, 128], F8)  # [p=e%128, echunk, h, t, key]
        QT8 = qt_pool.tile([128, 8, 1024], F8)     # [p=e%128, echunk, qcol]
        V8 = v_pool.tile([128, 2, 8, D], F8)       # [p=key%128, h, t, e]
        Vb = v_pool.tile([128, 2, D], BF16)        # bf16 V true kt0/1

        # DRAM bounce buffers for the pair exchanges
        stK = dram.tile([128, 8, 1024], F8)        # own K^T half
        gK = dram.tile([2, 128, 8, 1024], F8)
        stV = dram.tile([128, 8, 1024], F8)        # own V half
        gV = dram.tile([2, 128, 8, 1024], F8)

        with ExitStack() as ph12:
            x8_pool = ph12.enter_context(tc.tile_pool(name="x8p", bufs=1))
            xt_pool = ph12.enter_context(tc.tile_pool(name="xtp", bufs=1))
            w_pool = ph12.enter_context(tc.tile_pool(name="wp", bufs=1))
            hf_pool = ph12.enter_context(tc.tile_pool(name="hf", bufs=1))
            ps_mm = ph12.enter_context(tc.tile_pool(name="ps_mm", bufs=8, space="PSUM"))

            # ---- input DMAs: ONE queue (sync/HWDGE), strict priority ----
            wk8_sb = w_pool.tile([128, 8, 1024], F8, tag="wk8")
            x8 = x8_pool.tile([128, 8, 8, 128], F8, tag="x8")  # [p,c,t,q]
            wq8_sb = w_pool.tile([128, 8, 1024], F8, tag="wq8")
            wv8_sb = w_pool.tile([128, 8, 1024], F8, tag="wv8")
            xtb = xt_pool.tile([128, 2, 8, 128], BF16, tag="xtb")
            wvb_sb = w_pool.tile([128, 8, 1024], BF16, tag="wvb")

            nc.sync.dma_start(out=wk8_sb[:, 0:2, :], in_=wk8[:, 0:2, :])
            nc.sync.dma_start(out=x8[:, 0:2], in_=x8_in[:, 0:2])
            nc.sync.dma_start(out=wk8_sb[:, 2:8, :], in_=wk8[:, 2:8, :])
            nc.sync.dma_start(out=x8[:, 2:8], in_=x8_in[:, 2:8])
            nc.sync.dma_start(out=wq8_sb, in_=wq8[:, :, :])
            nc.sync.dma_start(out=wv8_sb, in_=wv8[:, :, :])
            nc.sync.dma_start(
                out=xtb, in_=xtb_in[:].rearrange("t p c q -> p t c q"))
            nc.sync.dma_start(out=wvb_sb, in_=wvb[:, :, :])
            nc.sync.dma_start(out=mask_sb, in_=mask_in[:, :])

            vhalf = hf_pool.tile([128, 8, 1024], F8, tag="vh")
            khalf = hf_pool.tile([128, 8, 1024], F8, tag="kh")

            def kq8_proj(w_sb, out_cb):
                # fp8 DoubleRow K^T/Q^T projection of all 8 local tiles:
                # stationary weight chunk-pair shared across both 4-tile
                # moving groups; out_cb(e, h, psum) consumes the result
                for e in range(8):
                    pps = [ps_mm.tile([128, 512], F32, tag="mm", name=f"p{e}_{h}")
                           for h in range(2)]
                    for c2 in range(4):
                        for h in range(2):
                            nc.tensor.matmul(
                                pps[h],
                                w_sb[:, 2 * c2:2 * c2 + 2, e * 128:(e + 1) * 128],
                                x8[:, 2 * c2:2 * c2 + 2, 4 * h:4 * h + 4, :],
                                start=(c2 == 0), stop=(c2 == 3),
                                perf_mode=DR,
                            )
                    for h in range(2):
                        out_cb(e, h, pps[h])

            def v_half():
                # fp8 DoubleRow; stationary x chunk-pair shared by both e-halves
                for lt in range(8):
                    vps = [ps_mm.tile([128, 512], F32, tag="mm", name=f"v{lt}_{eh}")
                           for eh in range(2)]
                    for c2 in range(4):
                        for eh in range(2):
                            nc.tensor.matmul(
                                vps[eh],
                                x8[:, 2 * c2:2 * c2 + 2, lt, :],
                                wv8_sb[:, 2 * c2:2 * c2 + 2, eh * 512:(eh + 1) * 512],
                                start=(c2 == 0), stop=(c2 == 3),
                                perf_mode=DR,
                            )
                    for eh in range(2):
                        nc.vector.tensor_copy(
                            vhalf[:, lt, eh * 512:(eh + 1) * 512], vps[eh])

            def vb_fix():
                # bf16 V of true tiles 0,1 (fixup)
                for t in range(2):
                    vbp = [ps_mm.tile([128, 512], F32, tag="mm", name=f"vb{t}_{eh}")
                           for eh in range(2)]
                    for c in range(8):
                        for eh in range(2):
                            nc.tensor.matmul(
                                vbp[eh], xtb[:, t, c, :],
                                wvb_sb[:, c, eh * 512:(eh + 1) * 512],
                                start=(c == 0), stop=(c == 7),
                            )
                    for eh in range(2):
                        nc.vector.tensor_copy(
                            Vb[:, t, eh * 512:(eh + 1) * 512], vbp[eh])

            # --- projections + pair exchange (collectives on gpsimd ring) ---
            # K first: S^T needs the gathered K^T earliest and the CC core
            # processes collectives strictly in issue order.
            kq8_proj(wk8_sb, lambda e, h, ps: nc.vector.tensor_copy(
                khalf[:, e, h * 512:(h + 1) * 512], ps))
            nc.gpsimd.dma_start(out=stK[:], in_=khalf)
            nc.gpsimd.collective_compute(
                "AllGather", mybir.AluOpType.bypass, replica_groups=PAIRS,
                ins=[stK.opt()], outs=[gK.opt()])
            # readback: gather index h = true key tiles {h, h+2, ...}
            for h in range(2):
                nc.sync.dma_start(
                    out=KT8[:, :, h, :, :],
                    in_=gK[h][:, :, :].rearrange("p e (t q) -> p e t q", q=128))

            def q_out(e, h, ps):
                nc.vector.tensor_copy(QT8[:, e, h * 512:(h + 1) * 512], ps)
            kq8_proj(wq8_sb, q_out)

            v_half()
            nc.gpsimd.dma_start(out=stV[:], in_=vhalf)
            nc.gpsimd.collective_compute(
                "AllGather", mybir.AluOpType.bypass, replica_groups=PAIRS,
                ins=[stV.opt()], outs=[gV.opt()])
            for h in range(2):
                nc.sync.dma_start(out=V8[:, h, :, :], in_=gV[h][:, :, :])
            vb_fix()

        # ---- attention: S^T per key tile, then AV with P^T stationary ----
        with ExitStack() as ph3:
            pt_pool = ph3.enter_context(tc.tile_pool(name="ptp", bufs=1))
            sc_pool = ph3.enter_context(tc.tile_pool(name="scp", bufs=2))
            outp = ph3.enter_context(tc.tile_pool(name="outp", bufs=2))

            PTs = [
                pt_pool.tile([128, 8, 512], F8, tag="pt1", name="PT1"),
                pt_pool.tile([128, 16, 512], F8, tag="pt2", name="PT2"),
            ]
            Pb = pt_pool.tile([128, 2, 128], BF16, tag="pb", name="Pb")

            def st_fused(ps_st):
                # one pass over key tiles; each KT stationary chunk-pair
                # serves BOTH slot groups' S^T matmuls (kt<8)
                for kt in range(16):
                    work = []   # (group, sps, w, col0, f)
                    for g in ((1, 0) if kt < 8 else (1,)):
                        Ls = LIMITS[g * 4:(g + 1) * 4]
                        f = sum(1 for L in Ls if L <= kt)
                        w = (4 - f) * 128
                        col0 = f * 128
                        sps = ps_st.tile([128, 512], F32, tag="st",
                                         name=f"s{g}_{kt}")
                        work.append((g, sps, w, col0, f))
                    for c2 in range(4):
                        for g, sps, w, col0, f in work:
                            nc.tensor.matmul(
                                sps[:, 0:w],
                                KT8[:, 2 * c2:2 * c2 + 2, kt % 2, kt // 2, :],
                                QT8[:, 2 * c2:2 * c2 + 2,
                                    g * 512 + col0: g * 512 + col0 + w],
                                start=(c2 == 0), stop=(c2 == 3),
                                perf_mode=DR,
                            )
                    for g, sps, w, col0, f in work:
                        Ls = LIMITS[g * 4:(g + 1) * 4]
                        if kt == Ls[f] - 2:
                            nc.vector.tensor_add(
                                sps[:, 0:128], sps[:, 0:128],
                                mask_sb[:, g * 256: g * 256 + 128],
                            )
                        elif kt == Ls[f] - 1:
                            nc.vector.tensor_add(
                                sps[:, 0:128], sps[:, 0:128],
                                mask_sb[:, g * 256 + 128: g * 256 + 256],
                            )
                        nc.scalar.activation(
                            PTs[g][:, kt, col0:col0 + w], sps[:, 0:w],
                            mybir.ActivationFunctionType.Exp,
                            bias=0.0, scale=EXP_SCALE8,
                        )
                        if g == 0 and kt < 2:
                            # bf16 P for the fixup slot (cols 0:128 = slot 0)
                            nc.scalar.activation(
                                Pb[:, kt, :], sps[:, 0:128],
                                mybir.ActivationFunctionType.Exp,
                                bias=0.0, scale=EXP_SCALE8,
                            )

            with ExitStack() as st_scope:
                ps_st = st_scope.enter_context(
                    tc.tile_pool(name="ps_st", bufs=3, space="PSUM"))
                st_fused(ps_st)

            ps_o = ph3.enter_context(tc.tile_pool(name="ps_o", bufs=3, space="PSUM"))
            ps_rs = ph3.enter_context(tc.tile_pool(name="ps_rs", bufs=1, space="PSUM"))

            def av_epilogue(slot, O_ps, rs_ps):
                stats = sc_pool.tile([128, 8], F32, tag="stats", name=f"st{slot}")
                recip = stats[:, 0:1]
                nc.vector.reciprocal(recip, rs_ps)
                out_sb = outp.tile([128, D], BF16, tag="osb", name=f"ou{slot}")
                nc.vector.tensor_scalar_mul(out_sb, O_ps, recip)
                eng = nc.scalar if slot % 2 == 0 else nc.gpsimd
                eng.dma_start(out=out_q[slot][:, :], in_=out_sb)

            def av_slot(g, j):
                # fp8 DoubleRow over key-tile pairs; rowsum reuses stationary
                PT = PTs[g]
                slot = g * 4 + j
                L = LIMITS[slot]
                col = j * 128
                O_ps = ps_o.tile([128, D], F32, tag="O", name=f"O{slot}")
                rs_ps = ps_rs.tile([128, 1], F32, tag="rs", name=f"r{slot}")
                L2 = L // 2
                for t2 in range(L2):
                    pt_blk = PT[:, 2 * t2:2 * t2 + 2, col:col + 128]
                    for h in range(2):
                        nc.tensor.matmul(
                            O_ps[:, h * 512:(h + 1) * 512], pt_blk,
                            V8[:, :, t2, h * 512:(h + 1) * 512],
                            start=(t2 == 0), stop=(t2 == L2 - 1),
                            perf_mode=DR,
                        )
                    nc.tensor.matmul(
                        rs_ps, pt_blk, ones8[:, :, 0:1],
                        start=(t2 == 0), stop=(t2 == L2 - 1),
                        perf_mode=DR,
                    )
                av_epilogue(slot, O_ps, rs_ps)

            def av_fix():
                # bf16 AV for the fixup slot (slot 0, L=2)
                O_ps = ps_o.tile([128, D], F32, tag="O", name="Ofix")
                rs_ps = ps_rs.tile([128, 1], F32, tag="rs", name="rfix")
                for kt in range(2):
                    pb_blk = Pb[:, kt, :]
                    for h in range(2):
                        nc.tensor.matmul(
                            O_ps[:, h * 512:(h + 1) * 512], pb_blk,
                            Vb[:, kt, h * 512:(h + 1) * 512],
                            start=(kt == 0), stop=(kt == 1),
                        )
                    nc.tensor.matmul(
                        rs_ps, pb_blk, onesb[:, 0:1],
                        start=(kt == 0), stop=(kt == 1),
                    )
                av_epilogue(0, O_ps, rs_ps)

            # interleave big(g1)/small(g0) slots in descending L; fixup slot
            # (L=2, bf16) last so the end-of-kernel chain is shortest.
            # st_fix sits late so the small bf16 gather (gb) has time to land.
            av_slot(1, 3)
            av_slot(0, 3)
            av_slot(1, 2)
            av_slot(0, 2)
            av_slot(1, 1)
            av_slot(0, 1)
            av_slot(1, 0)
            av_fix()

    nc.compile()
    return nc


def _masks():
    k = np.arange(128)[:, None]
    q = np.arange(128)[None, :]
    tril_t = np.where(k <= q, 0.0, NEG).astype(np.float32)  # S^T diag block
    fullneg = np.full((128, 128), NEG, np.float32)
    zeros = np.zeros((128, 128), np.float32)
    # slot j covers q-tile 2j+s with L=2j+2 key tiles: on s=0 the diagonal
    # is at kt=L-2 (kt=L-1 fully future); on s=1 kt=L-2 is fully attended
    # and the diagonal is at kt=L-1.  Same pattern for both groups.
    m_s0 = np.concatenate([tril_t, fullneg, tril_t, fullneg], axis=1)
    m_s1 = np.concatenate([zeros, tril_t, zeros, tril_t], axis=1)
    return m_s0, m_s1


def kernel(x, Wq, Wk, Wv):
    global LAST_EXEC_NS
    x = np.asarray(x, dtype=np.float32)
    Wq = np.asarray(Wq, dtype=np.float32)
    Wk = np.asarray(Wk, dtype=np.float32)
    Wv = np.asarray(Wv, dtype=np.float32)

    if "nc" not in _NC_CACHE:
        _NC_CACHE["nc"] = _build_nc()
    nc = _NC_CACHE["nc"]

    # host pre-transpose: x[b] (N, D) -> [tile, p=d%128, c=d//128, token]
    xt_f32 = np.ascontiguousarray(
        x.reshape(B, N_KTILES, 128, 8, 128).transpose(0, 1, 4, 3, 2)
    )  # [B, tile, p, c, q] f32
    x8_all = xt_f32.astype(F8NP)
    xtb_all = xt_f32.astype(BF)

    # weights [p=d%128, dchunk, ecol]; premuls folded in
    def wprep(W, premul, dt):
        return np.ascontiguousarray(
            (premul * W).reshape(8, 128, 1024).transpose(1, 0, 2).astype(dt))
    wq8_r = wprep(Wq, PRE8, F8NP)
    wk8_r = wprep(Wk, PRE8, F8NP)
    wv8_r = wprep(Wv, PRE8, F8NP)
    wvb_r = wprep(Wv, PRE8, BF)

    m_s0, m_s1 = _masks()
    in_maps = []
    for c in range(N_CORES):
        b, s = divmod(c, 2)
        # own tiles {s, s+2, ...} permuted to [p, c, t, q]
        x8_core = np.ascontiguousarray(
            x8_all[b, s::2].transpose(1, 2, 0, 3))
        in_maps.append({
            "x8_in": x8_core,
            "xtb": np.ascontiguousarray(xtb_all[b, 0:2]),
            "wq8": wq8_r, "wk8": wk8_r, "wv8": wv8_r,
            "wvb": wvb_r,
            "mask": m_s1 if s else m_s0,
        })

    res = run_bass_kernel_spmd(nc, in_maps, list(range(N_CORES)), trace=TRACE)
    LAST_EXEC_NS = res.exec_time_ns

    out = np.empty((B, N, D), dtype=np.float32)
    for c in range(N_CORES):
        b, s = divmod(c, 2)
        oq = np.asarray(res.results[c]["out_q"], dtype=np.float32)
        for j in range(N_SLOTS):
            g = 2 * j + s
            out[b, g * 128:(g + 1) * 128, :] = oq[j]
    return out


# revision 31
# speedup vs baseline: 1.0831x; 1.0037x over previous
"""Causal attention (B=4, N=2048, D=1024) on 8 Trainium2 NeuronCores.

v6 design (vs v5, 145us):
  * ALL THREE projections in fp8(e4m3) DoubleRow (Q and K too, not just
    V).  numpy-sim of the exact mix: quantization noise on q,k enters
    the softmax as a ~1.4% weight perturbation which stays at 4.4e-3
    max rel err (tolerance 2e-2) PROVIDED the early-row fixup computes
    its scores from true bf16 projections.  Weights premul 32 so fp8
    values sit in the normal range (|32q| < ~100 < 240 max normal);
    exp scale = (1/32)/1024 on the main path, (1/32)/16 on the fixup.
  * Parity resharding: core 2b+s owns the EVEN/ODD key tiles of batch b
    and the SAME set as query tiles -- one fp8 x upload (1 MB, layout
    [p, dchunk, tile, token]) feeds K, Q and V projections.  Slot j
    covers q-tile 2j+s with uniform limit L=2j+2; mask data kills the
    one extra future tile on s=0 cores.
  * Intra-pair split: each core projects only its 8 key tiles of K^T
    and V, exchanged via pair AllGathers (DRAM bounce, gpsimd ring).
    Gather index h = true key tiles {h, h+2, ...}; stepped-slice
    readbacks keep the program SPMD-uniform.
  * Early-row fixup: the L=2 slot (q-tile s) runs a full bf16 path for
    true keys 0..255: bf16 projections of the core's local tile 0
    (= true tile s) for K^T/V (pair-gathered) and Q^T (local).
  * All input DMAs on ONE sync/HWDGE queue in priority order; first
    matmul needs only ~0.4 MB.  Collective order: warmup (hides the
    first-begin latency behind the auto all-8 preamble barrier), gK,
    gV, gb.
"""
import sys

sys.path.insert(0, "/opt/trn_rl_repo")

from contextlib import ExitStack

import numpy as np
import ml_dtypes

import concourse.bass as bass
import concourse.mybir as mybir
import concourse.tile as tile
from concourse import bacc
from concourse.bass_utils import run_bass_kernel_spmd

B, N, D = 4, 2048, 1024
N_CORES = 8
N_SLOTS = 8
N_KTILES = 16
SCALE = 1.0 / 32.0      # 1/sqrt(D)
PRE8 = 32.0             # premul folded into all fp8 weights
PREB = 4.0              # premul folded into the bf16 fixup wq/wk
EXP_SCALE8 = SCALE / (PRE8 * PRE8)
EXP_SCALEB = SCALE / (PREB * PREB)
NEG = -1.0e9

F32 = mybir.dt.float32
BF16 = mybir.dt.bfloat16
F8 = mybir.dt.float8e4
DR = mybir.MatmulPerfMode.DoubleRow
BF = ml_dtypes.bfloat16
F8NP = ml_dtypes.float8_e4m3

PAIRS = [[0, 1], [2, 3], [4, 5], [6, 7]]

# uniform program limits per slot (key tiles 0..L-1 computed);
# slot j on core parity s covers q-tile 2j+s
LIMITS = [2, 4, 6, 8, 10, 12, 14, 16]

_NC_CACHE = {}
TRACE = False
LAST_EXEC_NS = None


def _build_nc():
    nc = bacc.Bacc(None, target_bir_lowering=False, debug=False, num_devices=8)

    # own-parity x tiles, fp8, host-permuted to [p=d%128, dchunk, tile, token]
    x8_in = nc.declare_dram_parameter("x8_in", [128, 8, 8, 128], F8, isOutput=False)
    # bf16 x of TRUE tiles 0,1 (same on both cores) for the K/V fixup
    # projections, and of the own q-tile s for the Q fixup
    xtb_in = nc.declare_dram_parameter("xtb", [2, 128, 8, 128], BF16, isOutput=False)
    # fp8 weights [p=d%128, dchunk, ecol] = e4m3(32*W)
    wq8 = nc.declare_dram_parameter("wq8", [128, 8, 1024], F8, isOutput=False)
    wk8 = nc.declare_dram_parameter("wk8", [128, 8, 1024], F8, isOutput=False)
    wv8 = nc.declare_dram_parameter("wv8", [128, 8, 1024], F8, isOutput=False)
    # bf16 fixup weight: wvb = 32*W (true-precision V for early rows)
    wvb = nc.declare_dram_parameter("wvb", [128, 8, 1024], BF16, isOutput=False)
    mask_in = nc.declare_dram_parameter("mask", [128, 512], F32, isOutput=False)
    out_q = nc.declare_dram_parameter("out_q", [N_SLOTS, 128, D], BF16, isOutput=True)

    with tile.TileContext(nc) as tc, ExitStack() as top:
        consts = top.enter_context(tc.tile_pool(name="consts", bufs=1))
        kt_pool = top.enter_context(tc.tile_pool(name="ktp", bufs=1))
        v_pool = top.enter_context(tc.tile_pool(name="vp", bufs=1))
        qt_pool = top.enter_context(tc.tile_pool(name="qtp", bufs=1))
        dram = top.enter_context(tc.tile_pool(name="dram", bufs=8, space="DRAM"))

        ones8 = consts.tile([128, 2, 16], F8)
        nc.vector.memset(ones8, PRE8)
        onesb = consts.tile([128, 8], BF16)
        nc.vector.memset(onesb, PRE8)
        mask_sb = consts.tile([128, 512], F32)

        # h-indexed so gather readbacks are CONTIGUOUS (true kt = 2t+h)
        KT8 = kt_pool.tile([128, 8, 2, 8, 128], F8)  # [p=e%128, echunk, h, t, key]
        QT8 = qt_pool.tile([128, 8, 1024], F8)     # [p=e%128, echunk, qcol]
        V8 = v_pool.tile([128, 2, 8, D], F8)       # [p=key%128, h, t, e]
        Vb = v_pool.tile([128, 2, D], BF16)        # bf16 V true kt0/1

        # DRAM bounce buffers for the pair exchanges
        stK = dram.tile([128, 8, 1024], F8)        # own K^T half
        gK = dram.tile([2, 128, 8, 1024], F8)
        stV = dram.tile([128, 8, 1024], F8)        # own V half
        gV = dram.tile([2, 128, 8, 1024], F8)

        with ExitStack() as ph12:
            x8_pool = ph12.enter_context(tc.tile_pool(name="x8p", bufs=1))
            xt_pool = ph12.enter_context(tc.tile_pool(name="xtp", bufs=1))
            w_pool = ph12.enter_context(tc.tile_pool(name="wp", bufs=1))
            hf_pool = ph12.enter_context(tc.tile_pool(name="hf", bufs=1))
            ps_mm = ph12.enter_context(tc.tile_pool(name="ps_mm", bufs=8, space="PSUM"))

            # ---- input DMAs: ONE queue (sync/HWDGE), strict priority ----
            wk8_sb = w_pool.tile([128, 8, 1024], F8, tag="wk8")
            x8 = x8_pool.tile([128, 8, 8, 128], F8, tag="x8")  # [p,c,t,q]
            wq8_sb = w_pool.tile([128, 8, 1024], F8, tag="wq8")
            wv8_sb = w_pool.tile([128, 8, 1024], F8, tag="wv8")
            xtb = xt_pool.tile([128, 2, 8, 128], BF16, tag="xtb")
            wvb_sb = w_pool.tile([128, 8, 1024], BF16, tag="wvb")

            nc.sync.dma_start(out=wk8_sb[:, 0:2, :], in_=wk8[:, 0:2, :])
            nc.sync.dma_start(out=x8[:, 0:2], in_=x8_in[:, 0:2])
            nc.sync.dma_start(out=wk8_sb[:, 2:8, :], in_=wk8[:, 2:8, :])
            nc.sync.dma_start(out=x8[:, 2:8], in_=x8_in[:, 2:8])
            nc.sync.dma_start(out=wq8_sb, in_=wq8[:, :, :])
            nc.sync.dma_start(out=wv8_sb, in_=wv8[:, :, :])
            nc.sync.dma_start(
                out=xtb, in_=xtb_in[:].rearrange("t p c q -> p t c q"))
            nc.sync.dma_start(out=wvb_sb, in_=wvb[:, :, :])
            nc.sync.dma_start(out=mask_sb, in_=mask_in[:, :])

            vhalf = hf_pool.tile([128, 8, 1024], F8, tag="vh")
            khalf = hf_pool.tile([128, 8, 1024], F8, tag="kh")

            def kq8_proj(w_sb, out_cb):
                # fp8 DoubleRow K^T/Q^T projection of all 8 local tiles:
                # stationary weight chunk-pair shared across both 4-tile
                # moving groups; out_cb(e, h, psum) consumes the result
                for e in range(8):
                    pps = [ps_mm.tile([128, 512], F32, tag="mm", name=f"p{e}_{h}")
                           for h in range(2)]
                    for c2 in range(4):
                        for h in range(2):
                            nc.tensor.matmul(
                                pps[h],
                                w_sb[:, 2 * c2:2 * c2 + 2, e * 128:(e + 1) * 128],
                                x8[:, 2 * c2:2 * c2 + 2, 4 * h:4 * h + 4, :],
                                start=(c2 == 0), stop=(c2 == 3),
                                perf_mode=DR,
                            )
                    for h in range(2):
                        out_cb(e, h, pps[h])

            def v_half():
                # fp8 DoubleRow; stationary x chunk-pair shared by both e-halves
                for lt in range(8):
                    vps = [ps_mm.tile([128, 512], F32, tag="mm", name=f"v{lt}_{eh}")
                           for eh in range(2)]
                    for c2 in range(4):
                        for eh in range(2):
                            nc.tensor.matmul(
                                vps[eh],
                                x8[:, 2 * c2:2 * c2 + 2, lt, :],
                                wv8_sb[:, 2 * c2:2 * c2 + 2, eh * 512:(eh + 1) * 512],
                                start=(c2 == 0), stop=(c2 == 3),
                                perf_mode=DR,
                            )
                    for eh in range(2):
                        nc.vector.tensor_copy(
                            vhalf[:, lt, eh * 512:(eh + 1) * 512], vps[eh])

            def vb_fix():
                # bf16 V of true tiles 0,1 (fixup)
                for t in range(2):
                    vbp = [ps_mm.tile([128, 512], F32, tag="mm", name=f"vb{t}_{eh}")
                           for eh in range(2)]
                    for c in range(8):
                        for eh in range(2):
                            nc.tensor.matmul(
                                vbp[eh], xtb[:, t, c, :],
                                wvb_sb[:, c, eh * 512:(eh + 1) * 512],
                                start=(c == 0), stop=(c == 7),
                            )
                    for eh in range(2):
                        nc.vector.tensor_copy(
                            Vb[:, t, eh * 512:(eh + 1) * 512], vbp[eh])

            # --- projections + pair exchange (collectives on gpsimd ring) ---
            # K first: S^T needs the gathered K^T earliest and the CC core
            # processes collectives strictly in issue order.
            kq8_proj(wk8_sb, lambda e, h, ps: nc.vector.tensor_copy(
                khalf[:, e, h * 512:(h + 1) * 512], ps))
            nc.gpsimd.dma_start(out=stK[:], in_=khalf)
            nc.gpsimd.collective_compute(
                "AllGather", mybir.AluOpType.bypass, replica_groups=PAIRS,
                ins=[stK.opt()], outs=[gK.opt()])
            # readback: gather index h = true key tiles {h, h+2, ...}
            for h in range(2):
                nc.sync.dma_start(
                    out=KT8[:, :, h, :, :],
                    in_=gK[h][:, :, :].rearrange("p e (t q) -> p e t q", q=128))

            def q_out(e, h, ps):
                nc.vector.tensor_copy(QT8[:, e, h * 512:(h + 1) * 512], ps)
            kq8_proj(wq8_sb, q_out)

            v_half()
            nc.gpsimd.dma_start(out=stV[:], in_=vhalf)
            nc.gpsimd.collective_compute(
                "AllGather", mybir.AluOpType.bypass, replica_groups=PAIRS,
                ins=[stV.opt()], outs=[gV.opt()])
            for h in range(2):
                nc.sync.dma_start(out=V8[:, h, :, :], in_=gV[h][:, :, :])
            vb_fix()

        # ---- attention: S^T per key tile, then AV with P^T stationary ----
        with ExitStack() as ph3:
            pt_pool = ph3.enter_context(tc.tile_pool(name="ptp", bufs=1))
            sc_pool = ph3.enter_context(tc.tile_pool(name="scp", bufs=2))
            outp = ph3.enter_context(tc.tile_pool(name="outp", bufs=2))

            PTs = [
                pt_pool.tile([128, 8, 512], F8, tag="pt1", name="PT1"),
                pt_pool.tile([128, 16, 512], F8, tag="pt2", name="PT2"),
            ]
            Pb = pt_pool.tile([128, 2, 128], BF16, tag="pb", name="Pb")

            def st_fused(ps_st):
                # one pass over key tiles; each KT stationary chunk-pair
                # serves BOTH slot groups' S^T matmuls (kt<8).  Even true
                # kts (readback h=0) first: h=1's readback lands later.
                for kt in list(range(0, 16, 2)) + list(range(1, 16, 2)):
                    work = []   # (group, sps, w, col0, f)
                    for g in ((1, 0) if kt < 8 else (1,)):
                        Ls = LIMITS[g * 4:(g + 1) * 4]
                        f = sum(1 for L in Ls if L <= kt)
                        w = (4 - f) * 128
                        col0 = f * 128
                        sps = ps_st.tile([128, 512], F32, tag="st",
                                         name=f"s{g}_{kt}")
                        work.append((g, sps, w, col0, f))
                    for c2 in range(4):
                        for g, sps, w, col0, f in work:
                            nc.tensor.matmul(
                                sps[:, 0:w],
                                KT8[:, 2 * c2:2 * c2 + 2, kt % 2, kt // 2, :],
                                QT8[:, 2 * c2:2 * c2 + 2,
                                    g * 512 + col0: g * 512 + col0 + w],
                                start=(c2 == 0), stop=(c2 == 3),
                                perf_mode=DR,
                            )
                    for g, sps, w, col0, f in work:
                        Ls = LIMITS[g * 4:(g + 1) * 4]
                        if kt == Ls[f] - 2:
                            nc.vector.tensor_add(
                                sps[:, 0:128], sps[:, 0:128],
                                mask_sb[:, g * 256: g * 256 + 128],
                            )
                        elif kt == Ls[f] - 1:
                            nc.vector.tensor_add(
                                sps[:, 0:128], sps[:, 0:128],
                                mask_sb[:, g * 256 + 128: g * 256 + 256],
                            )
                        nc.scalar.activation(
                            PTs[g][:, kt, col0:col0 + w], sps[:, 0:w],
                            mybir.ActivationFunctionType.Exp,
                            bias=0.0, scale=EXP_SCALE8,
                        )
                        if g == 0 and kt < 2:
                            # bf16 P for the fixup slot (cols 0:128 = slot 0)
                            nc.scalar.activation(
                                Pb[:, kt, :], sps[:, 0:128],
                                mybir.ActivationFunctionType.Exp,
                                bias=0.0, scale=EXP_SCALE8,
                            )

            with ExitStack() as st_scope:
                ps_st = st_scope.enter_context(
                    tc.tile_pool(name="ps_st", bufs=3, space="PSUM"))
                st_fused(ps_st)

            ps_o = ph3.enter_context(tc.tile_pool(name="ps_o", bufs=3, space="PSUM"))
            ps_rs = ph3.enter_context(tc.tile_pool(name="ps_rs", bufs=1, space="PSUM"))

            def av_epilogue(slot, O_ps, rs_ps):
                stats = sc_pool.tile([128, 8], F32, tag="stats", name=f"st{slot}")
                recip = stats[:, 0:1]
                nc.vector.reciprocal(recip, rs_ps)
                out_sb = outp.tile([128, D], BF16, tag="osb", name=f"ou{slot}")
                eng = nc.scalar if slot % 2 == 0 else nc.gpsimd
                for hh in range(2):
                    nc.vector.tensor_scalar_mul(
                        out_sb[:, hh * 512:(hh + 1) * 512],
                        O_ps[:, hh * 512:(hh + 1) * 512], recip)
                    eng.dma_start(
                        out=out_q[slot][:, hh * 512:(hh + 1) * 512],
                        in_=out_sb[:, hh * 512:(hh + 1) * 512])

            def av_slot(g, j):
                # fp8 DoubleRow over key-tile pairs; rowsum reuses stationary
                PT = PTs[g]
                slot = g * 4 + j
                L = LIMITS[slot]
                col = j * 128
                O_ps = ps_o.tile([128, D], F32, tag="O", name=f"O{slot}")
                rs_ps = ps_rs.tile([128, 1], F32, tag="rs", name=f"r{slot}")
                L2 = L // 2
                for t2 in range(L2):
                    pt_blk = PT[:, 2 * t2:2 * t2 + 2, col:col + 128]
                    for h in range(2):
                        nc.tensor.matmul(
                            O_ps[:, h * 512:(h + 1) * 512], pt_blk,
                            V8[:, :, t2, h * 512:(h + 1) * 512],
                            start=(t2 == 0), stop=(t2 == L2 - 1),
                            perf_mode=DR,
                        )
                    nc.tensor.matmul(
                        rs_ps, pt_blk, ones8[:, :, 0:1],
                        start=(t2 == 0), stop=(t2 == L2 - 1),
                        perf_mode=DR,
                    )
                av_epilogue(slot, O_ps, rs_ps)

            def av_fix():
                # bf16 AV for the fixup slot (slot 0, L=2)
                O_ps = ps_o.tile([128, D], F32, tag="O", name="Ofix")
                rs_ps = ps_rs.tile([128, 1], F32, tag="rs", name="rfix")
                for kt in range(2):
                    pb_blk = Pb[:, kt, :]
                    for h in range(2):
                        nc.tensor.matmul(
                            O_ps[:, h * 512:(h + 1) * 512], pb_blk,
                            Vb[:, kt, h * 512:(h + 1) * 512],
                            start=(kt == 0), stop=(kt == 1),
                        )
                    nc.tensor.matmul(
                        rs_ps, pb_blk, onesb[:, 0:1],
                        start=(kt == 0), stop=(kt == 1),
                    )
                av_epilogue(0, O_ps, rs_ps)

            # interleave big(g1)/small(g0) slots in descending L; fixup slot
            # (L=2, bf16) last so the end-of-kernel chain is shortest.
            # st_fix sits late so the small bf16 gather (gb) has time to land.
            av_slot(1, 3)
            av_slot(0, 3)
            av_slot(1, 2)
            av_slot(0, 2)
            av_slot(1, 1)
            av_slot(0, 1)
            av_slot(1, 0)
            av_fix()

    nc.compile()
    return nc


def _masks():
    k = np.arange(128)[:, None]
    q = np.arange(128)[None, :]
    tril_t = np.where(k <= q, 0.0, NEG).astype(np.float32)  # S^T diag block
    fullneg = np.full((128, 128), NEG, np.float32)
    zeros = np.zeros((128, 128), np.float32)
    # slot j covers q-tile 2j+s with L=2j+2 key tiles: on s=0 the diagonal
    # is at kt=L-2 (kt=L-1 fully future); on s=1 kt=L-2 is fully attended
    # and the diagonal is at kt=L-1.  Same pattern for both groups.
    m_s0 = np.concatenate([tril_t, fullneg, tril_t, fullneg], axis=1)
    m_s1 = np.concatenate([zeros, tril_t, zeros, tril_t], axis=1)
    return m_s0, m_s1


def kernel(x, Wq, Wk, Wv):
    global LAST_EXEC_NS
    x = np.asarray(x, dtype=np.float32)
    Wq = np.asarray(Wq, dtype=np.float32)
    Wk = np.asarray(Wk, dtype=np.float32)
    Wv = np.asarray(Wv, dtype=np.float32)

    if "nc" not in _NC_CACHE:
        _NC_CACHE["nc"] = _build_nc()
    nc = _NC_CACHE["nc"]

    # host pre-transpose: x[b] (N, D) -> [tile, p=d%128, c=d//128, token]
    xt_f32 = np.ascontiguousarray(
        x.reshape(B, N_KTILES, 128, 8, 128).transpose(0, 1, 4, 3, 2)
    )  # [B, tile, p, c, q] f32
    x8_all = xt_f32.astype(F8NP)
    xtb_all = xt_f32.astype(BF)

    # weights [p=d%128, dchunk, ecol]; premuls folded in
    def wprep(W, premul, dt):
        return np.ascontiguousarray(
            (premul * W).reshape(8, 128, 1024).transpose(1, 0, 2).astype(dt))
    wq8_r = wprep(Wq, PRE8, F8NP)
    wk8_r = wprep(Wk, PRE8, F8NP)
    wv8_r = wprep(Wv, PRE8, F8NP)
    wvb_r = wprep(Wv, PRE8, BF)

    m_s0, m_s1 = _masks()
    in_maps = []
    for c in range(N_CORES):
        b, s = divmod(c, 2)
        # own tiles {s, s+2, ...} permuted to [p, c, t, q]
        x8_core = np.ascontiguousarray(
            x8_all[b, s::2].transpose(1, 2, 0, 3))
        in_maps.append({
            "x8_in": x8_core,
            "xtb": np.ascontiguousarray(xtb_all[b, 0:2]),
            "wq8": wq8_r, "wk8": wk8_r, "wv8": wv8_r,
            "wvb": wvb_r,
            "mask": m_s1 if s else m_s0,
        })

    res = run_bass_kernel_spmd(nc, in_maps, list(range(N_CORES)), trace=TRACE)
    LAST_EXEC_NS = res.exec_time_ns

    out = np.empty((B, N, D), dtype=np.float32)
    for c in range(N_CORES):
        b, s = divmod(c, 2)
        oq = np.asarray(res.results[c]["out_q"], dtype=np.float32)
        for j in range(N_SLOTS):
            g = 2 * j + s
            out[b, g * 128:(g + 1) * 128, :] = oq[j]
    return out
